# revision 1
# baseline (speedup 1.0000x reference)
"""DenseEdgeOnlyConv kernel for 8 Trainium2 NeuronCores (Bass).

Reference computation (per node i, K=32 neighbors j=edge_index[i,k]):
    out[i] = mean_k relu((x[j] - x[i]) @ W + b)

Algebraic restructure: with u = x @ W, (x_j - x_i) @ W = u_j - u_i, so
    out[i] = (1/K) * sum_k relu(u_j - (u_i - b))
Per-edge matmuls become one [N,D]@[D,H] matmul plus a row gather of u.

Per-core plan (nodes sharded 6250/core, u table replicated per core):
  Phase A: compute u = x @ W into a private HBM table (f32, 512B rows).
  Phase B: per 128-node block, dma_gather the K neighbor rows of u
    (two calls: table rows < 32768 and >= 32768, since gather indices are
    int16), subtract u_i (broadcast), relu*(1/K) on ACT, reduce over K on DVE.

Host-side (sharding prep only): per-core node renumbering so each core's own
nodes occupy table rows [0, 6272) sorted by lo-edge count (makes per-block
gather column counts uniform across cores), per-node edge reordering
(lo-targets first; mean over K is permutation invariant), int16 index
tokens, and padding via table rows filled with -1e30 so pads contribute
exactly 0 through relu.
"""

import os
import sys

sys.path.insert(0, "/opt/trn_rl_repo")

import numpy as np

# problem constants (hardcoded per harness contract)
N, K, D, H = 50000, 32, 128, 128
NCORES = 8
OWN = N // NCORES            # 6250 real nodes per core
NB = 49                      # node blocks per core
SLOTS = NB * 128             # 6272 node slots per core (incl. 22 dummies)
PAD_LO_ROW = SLOTS           # table row holding -BIG (lo range)
OTH_BASE = SLOTS + 1         # first table row for other cores' nodes
LO = 32768                   # int16-addressable row limit per gather call
NOTH = N - OWN               # 43750
OTH_CHUNKS = 343             # 343*128 = 43904 >= NOTH
NCHUNK = NB + OTH_CHUNKS     # 392 u-compute chunks of 128 rows
R_TABLE = OTH_BASE + OTH_CHUNKS * 128 + 1   # 50178; last row = -BIG (hi range)
PAD_HI_REL = R_TABLE - 1 - LO               # 17409
NXROWS = NCHUNK * 128        # 50176 rows in per-core permuted x input
BIG_NEG = np.float32(-1e30)

LAST_RESULTS = None  # BassKernelResults of the last run (for profiling)


# ---------------------------------------------------------------- host prep

def _prep(edge_index):
    """Per-core renumbering, edge packing, and the global block schedule.

    Returns dict with per-core arrays and the shared (LS, HS) schedule.
    """
    cores = []
    for c in range(NCORES):
        lo_id, hi_id = OWN * c, OWN * (c + 1)
        own_ids = np.arange(lo_id, hi_id)
        row_of = np.full(N, -1, np.int64)
        oth = np.concatenate([np.arange(0, lo_id), np.arange(hi_id, N)])
        row_of[oth] = OTH_BASE + np.arange(NOTH)

        tgt = np.asarray(edge_index[lo_id:hi_id], dtype=np.int64)  # [OWN, K]
        own_mask = (tgt >= lo_id) & (tgt < hi_id)
        # lo-ness: own targets always lo (rows < 6272); others per row_of
        lo_mask0 = own_mask | (np.where(own_mask, OTH_BASE, row_of[tgt]) < LO)
        L = lo_mask0.sum(1)
        order = np.argsort(L, kind="stable")        # ascending lo-count
        sorted_ids = own_ids[order]
        row_of[sorted_ids] = np.arange(OWN)         # own rows = sorted slots

        rows = row_of[tgt]                          # final rows, [OWN, K]
        lo_mask = rows < LO
        ordk = np.argsort(~lo_mask, axis=1, kind="stable")   # lo targets first
        packed = np.take_along_axis(rows, ordk, axis=1)[order]  # [OWN, K]
        Ls = L[order]

        packed_full = np.zeros((SLOTS, K), np.int64)
        packed_full[:OWN] = packed
        VL = np.zeros(SLOTS, np.int64)
        VL[:OWN] = Ls
        VH = np.zeros(SLOTS, np.int64)
        VH[:OWN] = K - Ls
        cores.append(dict(sorted_ids=sorted_ids, packed=packed_full,
                          VL=VL, VH=VH))

    # global block schedule (shared across cores -> one SPMD program)
    LS = np.zeros(NB, np.int64)
    HS = np.zeros(NB, np.int64)
    for r in range(NB):
        sl = slice(128 * r, 128 * (r + 1))
        LS[r] = max(int(co["VL"][sl].max()) for co in cores)
        HS[r] = max(int(co["VH"][sl].max()) for co in cores)

    # per-core int16 token arrays, concatenated [128, ICOLS]
    off_lo = np.zeros(NB, np.int64)
    off_hi = np.zeros(NB, np.int64)
    off = 0
    for r in range(NB):
        off_lo[r] = off
        off += 8 * int(LS[r])
        off_hi[r] = off
        off += 8 * int(HS[r])
    icols = off

    for co in cores:
        idx_all = np.zeros((128, icols), np.int16)
        flat_lo_dbg = []
        flat_hi_dbg = []
        for r in range(NB):
            sl = slice(128 * r, 128 * (r + 1))
            rb = co["packed"][sl]                  # [128, K]
            vl, vh = co["VL"][sl], co["VH"][sl]
            LSr, HSr = int(LS[r]), int(HS[r])
            if LSr:
                kg = np.arange(LSr)[:, None]
                lo_tok = np.where(kg < vl[None, :], rb.T[:LSr, :], PAD_LO_ROW)
            else:
                lo_tok = np.zeros((0, 128), np.int64)
            if HSr:
                kg = np.arange(HSr)[:, None]
                gidx = np.minimum(vl[None, :] + kg, K - 1)
                vals = np.take_along_axis(rb, gidx.T, axis=1).T
                hi_tok = np.where(kg < vh[None, :], vals - LO, PAD_HI_REL)
            else:
                hi_tok = np.zeros((0, 128), np.int64)
            flat_lo = lo_tok.reshape(-1)           # k-major tokens
            flat_hi = hi_tok.reshape(-1)
            flat_lo_dbg.append(flat_lo)
            flat_hi_dbg.append(flat_hi)
            if LSr:
                w = flat_lo.reshape(-1, 16).T.astype(np.int16)  # [16, 8*LSr]
                idx_all[:, off_lo[r]:off_lo[r] + 8 * LSr] = np.tile(w, (8, 1))
            if HSr:
                w = flat_hi.reshape(-1, 16).T.astype(np.int16)
                idx_all[:, off_hi[r]:off_hi[r] + 8 * HSr] = np.tile(w, (8, 1))
        co["idx"] = idx_all
        co["flat_lo"] = flat_lo_dbg
        co["flat_hi"] = flat_hi_dbg

    return cores, LS, HS, off_lo, off_hi, icols


def _xperm(x, co):
    """Per-core permuted x input [NXROWS, D] (own sorted, zeros, others)."""
    sorted_ids = co["sorted_ids"]
    xp = np.zeros((NXROWS, D), np.float32)
    xp[:OWN] = x[sorted_ids]
    # others in ascending global id order (matches row_of assignment)
    lo_id = (sorted_ids // OWN)[0] * OWN
    oth = np.concatenate([np.arange(0, lo_id), np.arange(lo_id + OWN, N)])
    xp[SLOTS:SLOTS + NOTH] = x[oth]
    return xp


def _xperm_t(x, co):
    """Per-chunk transposed x input [128, NXROWS]: [d, 128*j + n] holds
    x_perm[128*j + n, d] (chunk j transposed for direct use as matmul lhsT)."""
    xp = _xperm(x, co)                       # [NXROWS, D]
    return np.ascontiguousarray(
        xp.reshape(NCHUNK, 128, D).transpose(0, 2, 1)    # [chunk, d, n]
        .transpose(1, 0, 2).reshape(D, NXROWS))


def _table_rows_for_chunk(j):
    """Table row range written by u-chunk j."""
    if j < NB:
        return 128 * j
    return OTH_BASE + 128 * (j - NB)


# ------------------------------------------------------- numpy device model

def _simulate_core(xp, W, b, co, LS, HS):
    """Numpy mirror of the device program (for host-logic validation)."""
    table = np.zeros((R_TABLE, H), np.float32)
    u = (xp.astype(np.float32) @ W.astype(np.float32)).astype(np.float32)
    for j in range(NCHUNK):
        r0 = _table_rows_for_chunk(j)
        table[r0:r0 + 128] = u[128 * j:128 * (j + 1)]
    table[PAD_LO_ROW] = BIG_NEG
    table[R_TABLE - 1] = BIG_NEG

    out = np.zeros((SLOTS, H), np.float32)
    for r in range(NB):
        LSr, HSr = int(LS[r]), int(HS[r])
        Cr = LSr + HSr
        T = np.zeros((128, Cr, H), np.float32)
        fl, fh = co["flat_lo"][r], co["flat_hi"][r]
        for i, t in enumerate(fl):
            T[i % 128, i // 128] = table[t]
        for i, t in enumerate(fh):
            T[i % 128, LSr + i // 128] = table[LO + t]
        ui = table[128 * r:128 * (r + 1)]          # [128, H]
        ui2 = ui - b[None, :].astype(np.float32)
        Dt = T - ui2[:, None, :]
        R = np.maximum(Dt * np.float32(1.0 / K), 0.0)
        out[128 * r:128 * (r + 1)] = R.sum(axis=1)
    return out


# ------------------------------------------------------------ bass program

def _build_program(LS, HS, off_lo, off_hi, icols, has_bias=True, repeat=1):
    import concourse.bacc as bacc
    import concourse.mybir as mybir
    from concourse.library_config import mlp
    from contextlib import ExitStack

    phase = os.environ.get("KERNEL_PHASE", "full")
    do_b = phase != "A"
    do_gather = phase == "full"
    glim = int(os.environ.get("KERNEL_GLIM", str(NB)))

    f32 = mybir.dt.float32
    CMAX = int((LS + HS).max())
    UIB = 4                     # ui tile buffers
    OTB = 4                     # o_t buffers

    # phase A DMA groups (chunk_start, nchunks); split at own/others boundary
    groups = []
    j = 0
    while j < NB:
        n = min(16, NB - j)
        groups.append((j, n))
        j += n
    while j < NCHUNK:
        n = min(16, NCHUNK - j)
        groups.append((j, n))
        j += n

    cumch = []
    t = 0
    for (_j0, n) in groups:
        t += n
        cumch.append(t)
    UST_FINAL = [sum(1 for g in range(len(groups)) if g % 2 == i) + 1
                 for i in (0, 1)]
    ECUM, OCUM = [], []
    e = o = 0
    qq = 0
    for (_j0, n) in groups:
        for _a in range(n):
            if qq % 2 == 0:
                e += 1
            else:
                o += 1
            qq += 1
        ECUM.append(e)
        OCUM.append(o)
    UI_FINAL = [sum(1 for r in range(NB) if r % 2 == i) for i in (0, 1)]

    nc = bacc.Bacc("TRN2")
    # xp_t: per-chunk transposed x — element [d, chunk*128 + n] = x_perm[row, d]
    xpt_d = nc.dram_tensor("xpt", [128, NXROWS], f32, kind="ExternalInput")
    W_d = nc.dram_tensor("Wm", [D, H], f32, kind="ExternalInput")
    brep_d = nc.dram_tensor("brep", [128, H], f32, kind="ExternalInput")
    neg_d = nc.dram_tensor("negrow", [1, H], f32, kind="ExternalInput")
    idx_d = nc.dram_tensor("idx", [128, icols], mybir.dt.int16,
                           kind="ExternalInput")
    out_d = nc.dram_tensor("out", [SLOTS, H], f32, kind="ExternalOutput")
    tab_d = nc.dram_tensor("utab", [R_TABLE, H], f32, kind="Internal")

    with ExitStack() as ctx:
        block = ctx.enter_context(nc.Block())
        f = mybir.dt.float32
        W_sb = ctx.enter_context(nc.sbuf_tensor("W_sb", [128, H], f))
        br_sb = ctx.enter_context(nc.sbuf_tensor("br_sb", [128, H], f))
        ng_sb = ctx.enter_context(nc.sbuf_tensor("ng_sb", [1, H], f))
        idx_sb = ctx.enter_context(
            nc.sbuf_tensor("idx_sb", [128, icols], mybir.dt.int16))
        xtg = ctx.enter_context(nc.sbuf_tensor("xtg", [128, 2, 16, 128], f))
        usb = ctx.enter_context(nc.sbuf_tensor("usb", [128, 2, 16, 128], f))
        T_t = ctx.enter_context(nc.sbuf_tensor("T_t", [128, 2, CMAX, 128], f))
        D_t = ctx.enter_context(nc.sbuf_tensor("D_t", [128, 2, CMAX, 128], f))
        ui = ctx.enter_context(nc.sbuf_tensor("ui", [128, UIB, 128], f))
        ui2 = ctx.enter_context(nc.sbuf_tensor("ui2", [128, UIB, 128], f))
        o_t = ctx.enter_context(nc.sbuf_tensor("o_t", [128, OTB, 128], f))
        ident = ctx.enter_context(nc.sbuf_tensor("ident", [128, 128], f))
        # one full 2KB PSUM bank per buffer: PE-write + other-engine-read in
        # the same bank is a hardware fault, so buffers must not share banks
        ps_u = [ctx.enter_context(nc.psum_tensor(f"ps_u{i}", [128, 512], f))
                for i in range(2)]
        ps_o = [ctx.enter_context(nc.psum_tensor(f"ps_o{i}", [128, 512], f))
                for i in range(2)]

        io = ctx.enter_context(nc.semaphore("io"))
        semsets = []
        for it in range(repeat):
            S = {}
            for nm in ("s_u", "s_uc", "s_uco", "s_uia", "s_sub", "s_rel",
                       "s_acc", "s_red"):
                S[nm] = ctx.enter_context(nc.semaphore(f"{nm}_{it}"))
            for nm in ("s_xin", "s_ust", "s_g"):
                S[nm] = [ctx.enter_context(nc.semaphore(f"{nm}{i}_{it}"))
                         for i in (0, 1)]
            S["s_ui"] = [ctx.enter_context(nc.semaphore(f"s_ui{i}_{it}"))
                         for i in range(UIB)]
            S["s_out"] = [ctx.enter_context(nc.semaphore(f"s_out{i}_{it}"))
                          for i in range(OTB)]
            semsets.append(S)

        # identity for the PE k-reduce: build once from iota/memset-free path:
        # load from brep? simplest: host provides it inside idx? -> use a
        # dedicated tiny input would add I/O; instead build with memset+iota
        # is awkward -> reuse neg trick: host already sends brep; identity is
        # cheap to send too but input count is fixed -- reuse "brep" slot is
        # wrong. Identity comes from DRAM input "identm".
        identm_d = nc.dram_tensor("identm", [128, 128], f32,
                                  kind="ExternalInput")

        cum_g = []
        tot = [0, 0]
        for r in range(NB):
            tot[r % 2] += (16 if LS[r] else 0) + (16 if HS[r] else 0)
            cum_g.append(tot[r % 2])

        @block.sync
        def _(sp):
            sp.dma_start(W_sb[:], W_d[:]).then_inc(io, 16)
            sp.dma_start(br_sb[:], brep_d[:]).then_inc(io, 16)
            sp.dma_start(ng_sb[:], neg_d[:]).then_inc(io, 16)
            sp.dma_start(idx_sb[:], idx_d[:]).then_inc(io, 16)
            sp.dma_start(ident[:], identm_d[:]).then_inc(io, 16)
            for it in range(repeat):
                S = semsets[it]
                s_xin, s_ust, s_ui, s_out = (S["s_xin"], S["s_ust"],
                                             S["s_ui"], S["s_out"])
                if it > 0:
                    P = semsets[it - 1]
                    if do_b:
                        sp.wait_ge(P["s_red"], NB)
                    for i in range(OTB):
                        sp.wait_ge(P["s_out"][i],
                                   16 * len([r for r in range(NB)
                                             if r % OTB == i]))

                def store_group(g):
                    j0, n = groups[g]
                    sp.wait_ge(S["s_uc"], ECUM[g])
                    sp.wait_ge(S["s_uco"], OCUM[g])
                    r0 = _table_rows_for_chunk(j0)
                    dst = tab_d[r0:r0 + 128 * n, :].rearrange(
                        "(a p) d -> p a d", p=128)
                    sp.dma_start(dst, usb[:, g % 2, 0:n, :]
                                 ).then_inc(s_ust[g % 2], 16)

                for g, (j0, n) in enumerate(groups):
                    if g >= 2:
                        # xtg[g%2] reuse: matmuls of group g-2 done
                        sp.wait_ge(S["s_u"], cumch[g - 2])
                    sp.dma_start(
                        xtg[:, g % 2, 0:n, :],
                        xpt_d[:, 128 * j0:128 * (j0 + n)].rearrange(
                            "p (a d) -> p a d", d=128)
                    ).then_inc(s_xin[g % 2], 16)
                    if g >= 1:
                        store_group(g - 1)
                store_group(len(groups) - 1)
                # pad rows
                sp.wait_ge(io, 80)
                sp.dma_start(tab_d[PAD_LO_ROW:PAD_LO_ROW + 1, :],
                             ng_sb[0:1, :]).then_inc(s_ust[0], 16)
                sp.dma_start(tab_d[R_TABLE - 1:R_TABLE, :],
                             ng_sb[0:1, :]).then_inc(s_ust[1], 16)
                # all table writes land before phase-B table reads (ui loads)
                sp.wait_ge(s_ust[0], 16 * UST_FINAL[0])
                sp.wait_ge(s_ust[1], 16 * UST_FINAL[1])
                # phase B: ui loads run ahead (UIB-deep); out stores trail
                nst = [0] * OTB
                for r in range(NB if do_b else 0):
                    if r >= UIB:
                        # ui[r%UIB] reuse: its last reader done
                        sp.wait_ge(S["s_uia"] if has_bias else S["s_sub"],
                                   r - UIB + 1)
                    sp.dma_start(ui[:, r % UIB, :],
                                 tab_d[128 * r:128 * (r + 1), :]
                                 ).then_inc(s_ui[r % UIB], 16)
                    if r >= 1:
                        sp.wait_ge(S["s_red"], r)
                        b = (r - 1) % OTB
                        sp.dma_start(out_d[128 * (r - 1):128 * r, :],
                                     o_t[:, b, :]).then_inc(s_out[b], 16)
                        nst[b] += 1
                if do_b:
                    sp.wait_ge(S["s_red"], NB)
                    b = (NB - 1) % OTB
                    sp.dma_start(out_d[128 * (NB - 1):128 * NB, :],
                                 o_t[:, b, :]).then_inc(s_out[b], 16)
                if not do_b:
                    for r in range(NB):
                        b = r % OTB
                        sp.dma_start(out_d[128 * r:128 * (r + 1), :],
                                     o_t[:, b, :]).then_inc(s_out[b], 16)

        @block.tensor
        def _(te):
            import concourse.mybir as mb
            te.wait_ge(io, 80)
            for it in range(repeat):
                S = semsets[it]
                s_xin = S["s_xin"]
                if it > 0:
                    if do_b:
                        te.wait_ge(semsets[it - 1]["s_red"], NB)
                    else:
                        te.wait_ge(semsets[it - 1]["s_uc"], (NCHUNK + 1) // 2)
                        te.wait_ge(semsets[it - 1]["s_uco"], NCHUNK // 2)
                q = 0
                for g, (j0, n) in enumerate(groups):
                    te.wait_ge(s_xin[g % 2], 16 * (g // 2 + 1))
                    for a in range(n):
                        if q >= 2:
                            # ps_u[q%2] reuse: its previous copy done
                            if q % 2 == 0:
                                te.wait_ge(S["s_uc"], q // 2)
                            else:
                                te.wait_ge(S["s_uco"], (q - 1) // 2)
                        nc.tensor.matmul(ps_u[q % 2][:, 0:128],
                                         xtg[:, g % 2, a, :],
                                         W_sb[:]).then_inc(S["s_u"], 1)
                        q += 1
                # phase B: k-reduce as identity-matmul accumulation
                for r in range(NB if do_b else 0):
                    Cr = int(LS[r] + HS[r])
                    te.wait_ge(S["s_rel"], r + 1)
                    if r >= 2:
                        te.wait_ge(S["s_red"], r - 1)      # ps_o[r%2] reuse
                    for k in range(Cr):
                        mm = nc.tensor.matmul(ps_o[r % 2][:, 0:128], ident[:],
                                              D_t[:, r % 2, k, :],
                                              start=(k == 0),
                                              stop=(k == Cr - 1))
                    mm.then_inc(S["s_acc"], 1)

        @block.scalar
        def _(sc):
            import concourse.mybir as mb
            sc.wait_ge(io, 80)
            for it in range(repeat):
                S = semsets[it]
                if it > 0:
                    if do_b:
                        sc.wait_ge(semsets[it - 1]["s_red"], NB)
                    else:
                        sc.wait_ge(semsets[it - 1]["s_uc"], (NCHUNK + 1) // 2)
                        sc.wait_ge(semsets[it - 1]["s_uco"], NCHUNK // 2)
                # phase A: odd-chunk psum->sbuf copies
                q = 0
                for g, (j0, n) in enumerate(groups):
                    first = True
                    for a in range(n):
                        if q % 2 == 1:
                            sc.wait_ge(S["s_u"], q + 1)
                            if first and g >= 2:
                                sc.wait_ge(S["s_ust"][g % 2], 16 * (g // 2))
                                first = False
                            nc.scalar.activation(
                                usb[:, g % 2, a, :], ps_u[q % 2][:, 0:128],
                                mb.ActivationFunctionType.Copy
                            ).then_inc(S["s_uco"], 1)
                        q += 1
                # phase B: relu * (1/K), in place
                for r in range(NB if do_b else 0):
                    sc.wait_ge(S["s_sub"], r + 1)
                    Cr = int(LS[r] + HS[r])
                    nc.scalar.activation(D_t[:, r % 2, 0:Cr, :],
                                         D_t[:, r % 2, 0:Cr, :],
                                         mb.ActivationFunctionType.Relu,
                                         scale=float(1.0 / K)
                                         ).then_inc(S["s_rel"], 1)

        @block.vector
        def _(ve):
            import concourse.mybir as mb
            ve.wait_ge(io, 80)
            for it in range(repeat):
                S = semsets[it]
                if it > 0:
                    if do_b:
                        ve.wait_ge(semsets[it - 1]["s_red"], NB)
                    else:
                        ve.wait_ge(semsets[it - 1]["s_uc"], (NCHUNK + 1) // 2)
                        ve.wait_ge(semsets[it - 1]["s_uco"], NCHUNK // 2)
                # phase A: even-chunk psum->sbuf copies
                q = 0
                for g, (j0, n) in enumerate(groups):
                    first = True
                    for a in range(n):
                        if q % 2 == 0:
                            ve.wait_ge(S["s_u"], q + 1)
                            if first and g >= 2:
                                # usb[g%2] reuse: store of group g-2 done
                                ve.wait_ge(S["s_ust"][g % 2], 16 * (g // 2))
                                first = False
                            nc.vector.tensor_copy(usb[:, g % 2, a, :],
                                                  ps_u[q % 2][:, 0:128]
                                                  ).then_inc(S["s_uc"], 1)
                        q += 1
                # phase B: subtract u_i (broadcast over cols); psum_o drain
                for r in range(NB if do_b else 0):
                    Cr = int(LS[r] + HS[r])
                    ve.wait_ge(S["s_ui"][r % UIB], 16 * (r // UIB + 1))
                    if has_bias:
                        nc.vector.tensor_tensor(ui2[:, r % UIB, :],
                                                ui[:, r % UIB, :],
                                                br_sb[:],
                                                mb.AluOpType.subtract
                                                ).then_inc(S["s_uia"], 1)
                        nc.vector.drain()
                        usrc = ui2
                    else:
                        usrc = ui   # b == 0: subtract u_i directly
                    if do_gather and r < glim:
                        ve.wait_ge(S["s_g"][r % 2], cum_g[r])
                    nc.vector.tensor_tensor(
                        D_t[:, r % 2, 0:Cr, :], T_t[:, r % 2, 0:Cr, :],
                        usrc[:, r % UIB, None, :].broadcast_to([128, Cr, 128]),
                        mb.AluOpType.subtract).then_inc(S["s_sub"], 1)
                    ve.wait_ge(S["s_acc"], r + 1)
                    if r >= OTB:
                        # o_t[r%OTB] reuse: store of block r-OTB done
                        ve.wait_ge(S["s_out"][r % OTB],
                                   16 * (r // OTB))
                    nc.vector.tensor_copy(o_t[:, r % OTB, :],
                                          ps_o[r % 2][:, 0:128]
                                          ).then_inc(S["s_red"], 1)

        @block.gpsimd
        def _(gp):
            if not do_gather:
                return
            gp.load_library(mlp)
            gp.wait_ge(io, 80)
            for it in range(repeat):
                S = semsets[it]
                s_ust, s_g = S["s_ust"], S["s_g"]
                if it > 0:
                    gp.wait_ge(semsets[it - 1]["s_red"], NB)
                gp.wait_ge(s_ust[0], 16 * UST_FINAL[0])
                gp.wait_ge(s_ust[1], 16 * UST_FINAL[1])
                for r in range(min(NB, glim)):
                    LSr, HSr = int(LS[r]), int(HS[r])
                    if r >= 2:
                        gp.wait_ge(S["s_sub"], r - 1)      # T_t[r%2] reuse
                    if LSr:
                        gp.dma_gather(
                            T_t[:, r % 2, 0:LSr, :], tab_d[0:LO, :],
                            idx_sb[:, off_lo[r]:off_lo[r] + 8 * LSr],
                            128 * LSr, 128 * LSr, H,
                            single_packet=False).then_inc(s_g[r % 2], 16)
                    if HSr:
                        gp.dma_gather(
                            T_t[:, r % 2, LSr:LSr + HSr, :],
                            tab_d[LO:R_TABLE, :],
                            idx_sb[:, off_hi[r]:off_hi[r] + 8 * HSr],
                            128 * HSr, 128 * HSr, H,
                            single_packet=False).then_inc(s_g[r % 2], 16)

    nc.compile()
    return nc


# ----------------------------------------------------------------- kernel()

def kernel(x, W, b, edge_index, _simulate=False):
    x = np.asarray(x, np.float32)
    W = np.asarray(W, np.float32)
    b = np.asarray(b, np.float32)
    edge_index = np.asarray(edge_index)

    cores, LS, HS, off_lo, off_hi, icols = _prep(edge_index)

    if _simulate:
        full = np.empty((N, H), np.float32)
        for c, co in enumerate(cores):
            out = _simulate_core(_xperm(x, co), W, b, co, LS, HS)
            full[co["sorted_ids"]] = out[:OWN]
        return full

    from concourse.bass_utils import run_bass_kernel_spmd

    has_bias = bool(np.any(b != 0))
    nc = _build_program(LS, HS, off_lo, off_hi, icols, has_bias)

    brep = np.tile(b[None, :], (128, 1)).astype(np.float32)
    ident = np.eye(128, dtype=np.float32)
    neg = np.full((1, H), BIG_NEG, np.float32)
    in_maps = []
    for co in cores:
        in_maps.append({
            "xpt": _xperm_t(x, co),
            "Wm": W,
            "brep": brep,
            "identm": ident,
            "negrow": neg,
            "idx": co["idx"],
        })

    trace = bool(int(os.environ.get("KERNEL_TRACE", "0")))
    res = run_bass_kernel_spmd(nc, in_maps, core_ids=list(range(NCORES)),
                               trace=trace)
    global LAST_RESULTS
    LAST_RESULTS = res

    full = np.empty((N, H), np.float32)
    for c, co in enumerate(cores):
        out = np.asarray(res.results[c]["out"], np.float32)
        full[co["sorted_ids"]] = out[:OWN]
    return full



# revision 2
# speedup vs baseline: 2.0098x; 2.0098x over previous
"""DenseEdgeOnlyConv kernel for 8 Trainium2 NeuronCores (Bass).

Reference computation (per node i, K=32 neighbors j=edge_index[i,k]):
    out[i] = mean_k relu((x[j] - x[i]) @ W + b)

Algebraic restructure: with u = x @ W, (x_j - x_i) @ W = u_j - u_i, so with
v = u_i - b:
    out[i] = (1/K) * sum_k relu(u_j - v)
           = (1/K) * (sum_k max(u_j, v) - K*v)
The max form needs no separate relu pass: pad gather slots point at a -BIG
table row, max(-BIG, v) = v, so a block with C >= K slots per node yields
    out[i] = (1/K) * (sum_{C slots} max(u_j, v) - C*v).

Per-core plan (nodes sharded 6250/core, u table replicated per core, bf16):
  Phase A: compute u = x @ W (bf16 in, bf16 out) into a private HBM table.
  Phase B: per 128-node block, dma_gather the neighbor rows of u in bf16
    (lo/hi split: rows < 32768 vs rest, since gather indices are int16),
    round-robining blocks over 4 SWDGE queues so descriptor generation runs
    on all 4 GpSimd Q7 core-pairs concurrently (~3.7x single-queue rate).
    DVE computes max(T, v) in bf16, PE accumulates the C column-blocks into
    PSUM via identity matmuls (bf16), DVE applies (psum - C*v)/K.

Host-side (sharding prep only): per-core node renumbering so each core's own
nodes occupy table rows [0, 6272) sorted by lo-edge count (makes per-block
gather column counts uniform across cores), per-node edge reordering
(lo-targets first; mean over K is permutation invariant), int16 index
tokens, and padding via table rows filled with -1e30 so pads contribute
exactly 0 through the max-trick.
"""

import os
import sys

sys.path.insert(0, "/opt/trn_rl_repo")

import numpy as np

# problem constants (hardcoded per harness contract)
N, K, D, H = 50000, 32, 128, 128
NCORES = 8
OWN = N // NCORES            # 6250 real nodes per core
NB = 49                      # node blocks per core
SLOTS = NB * 128             # 6272 node slots per core (incl. 22 dummies)
PAD_LO_ROW = SLOTS           # table row holding -BIG (lo range)
OTH_BASE = SLOTS + 1         # first table row for other cores' nodes
LO = 32768                   # int16-addressable row limit per gather call
NOTH = N - OWN               # 43750
OTH_CHUNKS = 343             # 343*128 = 43904 >= NOTH
NCHUNK = NB + OTH_CHUNKS     # 392 u-compute chunks of 128 rows
R_TABLE = OTH_BASE + OTH_CHUNKS * 128 + 1   # 50178; last row = -BIG (hi range)
PAD_HI_REL = R_TABLE - 1 - LO               # 17409
NXROWS = NCHUNK * 128        # 50176 rows in per-core permuted x input
BIG_NEG = np.float32(-1e30)
NQ = 4                       # SWDGE queues (= concurrent Q7 DGE core pairs)
TTB = 4                      # T_t buffers (one per in-flight gather block)

LAST_RESULTS = None  # BassKernelResults of the last run (for profiling)


# ---------------------------------------------------------------- host prep

def _prep(edge_index):
    """Per-core renumbering, edge packing, and the global block schedule.

    Returns dict with per-core arrays and the shared (LS, HS) schedule.
    """
    cores = []
    for c in range(NCORES):
        lo_id, hi_id = OWN * c, OWN * (c + 1)
        own_ids = np.arange(lo_id, hi_id)
        row_of = np.full(N, -1, np.int64)
        oth = np.concatenate([np.arange(0, lo_id), np.arange(hi_id, N)])
        row_of[oth] = OTH_BASE + np.arange(NOTH)

        tgt = np.asarray(edge_index[lo_id:hi_id], dtype=np.int64)  # [OWN, K]
        own_mask = (tgt >= lo_id) & (tgt < hi_id)
        # lo-ness: own targets always lo (rows < 6272); others per row_of
        lo_mask0 = own_mask | (np.where(own_mask, OTH_BASE, row_of[tgt]) < LO)
        L = lo_mask0.sum(1)
        order = np.argsort(L, kind="stable")        # ascending lo-count
        sorted_ids = own_ids[order]
        row_of[sorted_ids] = np.arange(OWN)         # own rows = sorted slots

        rows = row_of[tgt]                          # final rows, [OWN, K]
        lo_mask = rows < LO
        ordk = np.argsort(~lo_mask, axis=1, kind="stable")   # lo targets first
        packed = np.take_along_axis(rows, ordk, axis=1)[order]  # [OWN, K]
        Ls = L[order]

        packed_full = np.zeros((SLOTS, K), np.int64)
        packed_full[:OWN] = packed
        VL = np.zeros(SLOTS, np.int64)
        VL[:OWN] = Ls
        VH = np.zeros(SLOTS, np.int64)
        VH[:OWN] = K - Ls
        cores.append(dict(sorted_ids=sorted_ids, packed=packed_full,
                          VL=VL, VH=VH))

    # global block schedule (shared across cores -> one SPMD program)
    LS = np.zeros(NB, np.int64)
    HS = np.zeros(NB, np.int64)
    for r in range(NB):
        sl = slice(128 * r, 128 * (r + 1))
        LS[r] = max(int(co["VL"][sl].max()) for co in cores)
        HS[r] = max(int(co["VH"][sl].max()) for co in cores)

    # per-core int16 token arrays, concatenated [128, ICOLS]
    off_lo = np.zeros(NB, np.int64)
    off_hi = np.zeros(NB, np.int64)
    off = 0
    for r in range(NB):
        off_lo[r] = off
        off += 8 * int(LS[r])
        off_hi[r] = off
        off += 8 * int(HS[r])
    icols = off

    for co in cores:
        idx_all = np.zeros((128, icols), np.int16)
        flat_lo_dbg = []
        flat_hi_dbg = []
        for r in range(NB):
            sl = slice(128 * r, 128 * (r + 1))
            rb = co["packed"][sl]                  # [128, K]
            vl, vh = co["VL"][sl], co["VH"][sl]
            LSr, HSr = int(LS[r]), int(HS[r])
            if LSr:
                kg = np.arange(LSr)[:, None]
                lo_tok = np.where(kg < vl[None, :], rb.T[:LSr, :], PAD_LO_ROW)
            else:
                lo_tok = np.zeros((0, 128), np.int64)
            if HSr:
                kg = np.arange(HSr)[:, None]
                gidx = np.minimum(vl[None, :] + kg, K - 1)
                vals = np.take_along_axis(rb, gidx.T, axis=1).T
                hi_tok = np.where(kg < vh[None, :], vals - LO, PAD_HI_REL)
            else:
                hi_tok = np.zeros((0, 128), np.int64)
            flat_lo = lo_tok.reshape(-1)           # k-major tokens
            flat_hi = hi_tok.reshape(-1)
            flat_lo_dbg.append(flat_lo)
            flat_hi_dbg.append(flat_hi)
            if LSr:
                w = flat_lo.reshape(-1, 16).T.astype(np.int16)  # [16, 8*LSr]
                idx_all[:, off_lo[r]:off_lo[r] + 8 * LSr] = np.tile(w, (8, 1))
            if HSr:
                w = flat_hi.reshape(-1, 16).T.astype(np.int16)
                idx_all[:, off_hi[r]:off_hi[r] + 8 * HSr] = np.tile(w, (8, 1))
        co["idx"] = idx_all
        co["flat_lo"] = flat_lo_dbg
        co["flat_hi"] = flat_hi_dbg

    return cores, LS, HS, off_lo, off_hi, icols


def _xperm(x, co):
    """Per-core permuted x input [NXROWS, D] (own sorted, zeros, others)."""
    sorted_ids = co["sorted_ids"]
    xp = np.zeros((NXROWS, D), np.float32)
    xp[:OWN] = x[sorted_ids]
    # others in ascending global id order (matches row_of assignment)
    lo_id = (sorted_ids // OWN)[0] * OWN
    oth = np.concatenate([np.arange(0, lo_id), np.arange(lo_id + OWN, N)])
    xp[SLOTS:SLOTS + NOTH] = x[oth]
    return xp


def _xperm_t(x, co):
    """Per-chunk transposed x input [128, NXROWS]: [d, 128*j + n] holds
    x_perm[128*j + n, d] (chunk j transposed for direct use as matmul lhsT)."""
    xp = _xperm(x, co)                       # [NXROWS, D]
    return np.ascontiguousarray(
        xp.reshape(NCHUNK, 128, D).transpose(0, 2, 1)    # [chunk, d, n]
        .transpose(1, 0, 2).reshape(D, NXROWS))


def _table_rows_for_chunk(j):
    """Table row range written by u-chunk j."""
    if j < NB:
        return 128 * j
    return OTH_BASE + 128 * (j - NB)


# ------------------------------------------------------- numpy device model

def _simulate_core(xp, W, b, co, LS, HS):
    """Numpy mirror of the device program (for host-logic validation)."""
    import ml_dtypes
    bf = ml_dtypes.bfloat16
    table = np.zeros((R_TABLE, H), np.float32)
    u = (xp.astype(bf).astype(np.float32)
         @ W.astype(bf).astype(np.float32)).astype(bf).astype(np.float32)
    for j in range(NCHUNK):
        r0 = _table_rows_for_chunk(j)
        table[r0:r0 + 128] = u[128 * j:128 * (j + 1)]
    table[PAD_LO_ROW] = BIG_NEG
    table[R_TABLE - 1] = BIG_NEG

    out = np.zeros((SLOTS, H), np.float32)
    for r in range(NB):
        LSr, HSr = int(LS[r]), int(HS[r])
        Cr = LSr + HSr
        T = np.zeros((128, Cr, H), np.float32)
        fl, fh = co["flat_lo"][r], co["flat_hi"][r]
        for i, t in enumerate(fl):
            T[i % 128, i // 128] = table[t]
        for i, t in enumerate(fh):
            T[i % 128, LSr + i // 128] = table[LO + t]
        ui = table[128 * r:128 * (r + 1)]          # [128, H], already bf16
        v = (ui - b[None, :]).astype(bf).astype(np.float32)
        M = np.maximum(T, v[:, None, :]).astype(bf).astype(np.float32)
        acc = M.sum(axis=1, dtype=np.float32)      # PE accumulates in f32
        out[128 * r:128 * (r + 1)] = (acc - Cr * v) * np.float32(1.0 / K)
    return out


# ------------------------------------------------------------ bass program

def _build_program(LS, HS, off_lo, off_hi, icols, has_bias=True):
    import concourse.bacc as bacc
    import concourse.mybir as mybir
    from concourse.library_config import mlp
    from contextlib import ExitStack

    f32 = mybir.dt.float32
    bf16 = mybir.dt.bfloat16
    CMAX = int((LS + HS).max())
    UIB = 4                     # ui tile buffers
    OTB = 4                     # o_t buffers
    VSB = 4                     # Vs buffers

    # phase A DMA groups (chunk_start, nchunks); split at own/others boundary
    groups = []
    j = 0
    while j < NB:
        n = min(16, NB - j)
        groups.append((j, n))
        j += n
    while j < NCHUNK:
        n = min(16, NCHUNK - j)
        groups.append((j, n))
        j += n

    cumch = []
    t = 0
    for (_j0, n) in groups:
        t += n
        cumch.append(t)
    UST_FINAL = [sum(1 for g in range(len(groups)) if g % 2 == i) + 1
                 for i in (0, 1)]
    ECUM, OCUM = [], []
    e = o = 0
    qq = 0
    for (_j0, n) in groups:
        for _a in range(n):
            if qq % 2 == 0:
                e += 1
            else:
                o += 1
            qq += 1
        ECUM.append(e)
        OCUM.append(o)

    nc = bacc.Bacc("TRN2", num_swdge_queues=NQ)
    # xp_t: per-chunk transposed x — element [d, chunk*128 + n] = x_perm[row, d]
    xpt_d = nc.dram_tensor("xpt", [128, NXROWS], bf16, kind="ExternalInput")
    W_d = nc.dram_tensor("Wm", [D, H], bf16, kind="ExternalInput")
    brep_d = nc.dram_tensor("brep", [128, H], bf16, kind="ExternalInput")
    neg_d = nc.dram_tensor("negrow", [1, H], bf16, kind="ExternalInput")
    identm_d = nc.dram_tensor("identm", [128, 128], bf16, kind="ExternalInput")
    idx_d = nc.dram_tensor("idx", [128, icols], mybir.dt.int16,
                           kind="ExternalInput")
    out_d = nc.dram_tensor("out", [SLOTS, H], f32, kind="ExternalOutput")
    tab_d = nc.dram_tensor("utab", [R_TABLE, H], bf16, kind="Internal")

    with ExitStack() as ctx:
        block = ctx.enter_context(nc.Block())
        W_sb = ctx.enter_context(nc.sbuf_tensor("W_sb", [128, H], bf16))
        br_sb = ctx.enter_context(nc.sbuf_tensor("br_sb", [128, H], bf16))
        ng_sb = ctx.enter_context(nc.sbuf_tensor("ng_sb", [1, H], bf16))
        idx_sb = ctx.enter_context(
            nc.sbuf_tensor("idx_sb", [128, icols], mybir.dt.int16))
        xtg = ctx.enter_context(nc.sbuf_tensor("xtg", [128, 2, 16, 128], bf16))
        usb = ctx.enter_context(nc.sbuf_tensor("usb", [128, 2, 16, 128], bf16))
        T_t = ctx.enter_context(
            nc.sbuf_tensor("T_t", [128, TTB, CMAX, 128], bf16))
        D_t = ctx.enter_context(
            nc.sbuf_tensor("D_t", [128, 2, CMAX, 128], bf16))
        ui = ctx.enter_context(nc.sbuf_tensor("ui", [128, UIB, 128], bf16))
        ui2 = ctx.enter_context(nc.sbuf_tensor("ui2", [128, UIB, 128], bf16))
        vs = ctx.enter_context(nc.sbuf_tensor("vs", [128, VSB, 128], f32))
        o_t = ctx.enter_context(nc.sbuf_tensor("o_t", [128, OTB, 128], f32))
        ident = ctx.enter_context(nc.sbuf_tensor("ident", [128, 128], bf16))
        # one full 2KB PSUM bank per buffer: PE-write + other-engine-read in
        # the same bank is a hardware fault, so buffers must not share banks
        ps_u = [ctx.enter_context(nc.psum_tensor(f"ps_u{i}", [128, 512], f32))
                for i in range(2)]
        ps_o = [ctx.enter_context(nc.psum_tensor(f"ps_o{i}", [128, 512], f32))
                for i in range(2)]

        io = ctx.enter_context(nc.semaphore("io"))
        S = {}
        for nm in ("s_u", "s_uc", "s_uco", "s_uia", "s_sub", "s_vs",
                   "s_acc", "s_red"):
            S[nm] = ctx.enter_context(nc.semaphore(nm))
        for nm in ("s_xin", "s_ust"):
            S[nm] = [ctx.enter_context(nc.semaphore(f"{nm}{i}"))
                     for i in (0, 1)]
        S["s_g"] = [ctx.enter_context(nc.semaphore(f"s_g{i}"))
                    for i in range(TTB)]
        S["s_ui"] = [ctx.enter_context(nc.semaphore(f"s_ui{i}"))
                     for i in range(UIB)]
        S["s_out"] = [ctx.enter_context(nc.semaphore(f"s_out{i}"))
                      for i in range(OTB)]

        @block.sync
        def _(sp):
            sp.dma_start(W_sb[:], W_d[:]).then_inc(io, 16)
            sp.dma_start(br_sb[:], brep_d[:]).then_inc(io, 16)
            sp.dma_start(ng_sb[:], neg_d[:]).then_inc(io, 16)
            sp.dma_start(idx_sb[:], idx_d[:]).then_inc(io, 16)
            sp.dma_start(ident[:], identm_d[:]).then_inc(io, 16)
            s_xin, s_ust, s_ui, s_out = (S["s_xin"], S["s_ust"],
                                         S["s_ui"], S["s_out"])

            def store_group(g):
                j0, n = groups[g]
                sp.wait_ge(S["s_uc"], ECUM[g])
                sp.wait_ge(S["s_uco"], OCUM[g])
                r0 = _table_rows_for_chunk(j0)
                dst = tab_d[r0:r0 + 128 * n, :].rearrange(
                    "(a p) d -> p a d", p=128)
                sp.dma_start(dst, usb[:, g % 2, 0:n, :]
                             ).then_inc(s_ust[g % 2], 16)

            for g, (j0, n) in enumerate(groups):
                if g >= 2:
                    # xtg[g%2] reuse: matmuls of group g-2 done
                    sp.wait_ge(S["s_u"], cumch[g - 2])
                sp.dma_start(
                    xtg[:, g % 2, 0:n, :],
                    xpt_d[:, 128 * j0:128 * (j0 + n)].rearrange(
                        "p (a d) -> p a d", d=128)
                ).then_inc(s_xin[g % 2], 16)
                if g >= 1:
                    store_group(g - 1)
            store_group(len(groups) - 1)
            # pad rows
            sp.wait_ge(io, 80)
            sp.dma_start(tab_d[PAD_LO_ROW:PAD_LO_ROW + 1, :],
                         ng_sb[0:1, :]).then_inc(s_ust[0], 16)
            sp.dma_start(tab_d[R_TABLE - 1:R_TABLE, :],
                         ng_sb[0:1, :]).then_inc(s_ust[1], 16)
            # all table writes land before phase-B table reads (ui loads)
            sp.wait_ge(s_ust[0], 16 * UST_FINAL[0])
            sp.wait_ge(s_ust[1], 16 * UST_FINAL[1])
            # phase B: ui loads run ahead (UIB-deep); out stores trail
            for r in range(NB):
                if r >= UIB:
                    # ui[r%UIB] reuse: its last reader done.  readers: DVE
                    # max-pass (s_sub) and ACT vs build (s_vs) [+ ui2 sub]
                    sp.wait_ge(S["s_sub"], r - UIB + 1)
                    sp.wait_ge(S["s_vs"], r - UIB + 1)
                sp.dma_start(ui[:, r % UIB, :],
                             tab_d[128 * r:128 * (r + 1), :]
                             ).then_inc(s_ui[r % UIB], 16)
                if r >= 1:
                    sp.wait_ge(S["s_red"], r)
                    b = (r - 1) % OTB
                    sp.dma_start(out_d[128 * (r - 1):128 * r, :],
                                 o_t[:, b, :]).then_inc(s_out[b], 16)
            sp.wait_ge(S["s_red"], NB)
            b = (NB - 1) % OTB
            sp.dma_start(out_d[128 * (NB - 1):128 * NB, :],
                         o_t[:, b, :]).then_inc(s_out[b], 16)

        @block.tensor
        def _(te):
            te.wait_ge(io, 80)
            s_xin = S["s_xin"]
            q = 0
            for g, (j0, n) in enumerate(groups):
                te.wait_ge(s_xin[g % 2], 16 * (g // 2 + 1))
                for a in range(n):
                    if q >= 2:
                        # ps_u[q%2] reuse: its previous copy done
                        if q % 2 == 0:
                            te.wait_ge(S["s_uc"], q // 2)
                        else:
                            te.wait_ge(S["s_uco"], (q - 1) // 2)
                    nc.tensor.matmul(ps_u[q % 2][:, 0:128],
                                     xtg[:, g % 2, a, :],
                                     W_sb[:]).then_inc(S["s_u"], 1)
                    q += 1
            # phase B: C-column accumulate as identity-matmul into PSUM
            for r in range(NB):
                Cr = int(LS[r] + HS[r])
                te.wait_ge(S["s_sub"], r + 1)
                if r >= 2:
                    te.wait_ge(S["s_red"], r - 1)      # ps_o[r%2] reuse
                for k in range(Cr):
                    mm = nc.tensor.matmul(ps_o[r % 2][:, 0:128], ident[:],
                                          D_t[:, r % 2, k, :],
                                          start=(k == 0),
                                          stop=(k == Cr - 1))
                mm.then_inc(S["s_acc"], 1)

        @block.scalar
        def _(sc):
            import concourse.mybir as mb
            sc.wait_ge(io, 80)
            # phase A: odd-chunk psum->sbuf copies (downcast to bf16)
            q = 0
            for g, (j0, n) in enumerate(groups):
                first = True
                for a in range(n):
                    if q % 2 == 1:
                        sc.wait_ge(S["s_u"], q + 1)
                        if first and g >= 2:
                            sc.wait_ge(S["s_ust"][g % 2], 16 * (g // 2))
                            first = False
                        nc.scalar.activation(
                            usb[:, g % 2, a, :], ps_u[q % 2][:, 0:128],
                            mb.ActivationFunctionType.Copy
                        ).then_inc(S["s_uco"], 1)
                    q += 1
            # phase B: vs[r] = v * (Cr/K) where v = ui (- b)
            for r in range(NB):
                Cr = int(LS[r] + HS[r])
                sc.wait_ge(S["s_ui"][r % UIB], 16 * (r // UIB + 1))
                if has_bias:
                    sc.wait_ge(S["s_uia"], r + 1)
                    vsrc = ui2
                else:
                    vsrc = ui
                if r >= VSB:
                    sc.wait_ge(S["s_red"], r - VSB + 1)   # vs[r%VSB] reuse
                nc.scalar.activation(vs[:, r % VSB, :], vsrc[:, r % UIB, :],
                                     mb.ActivationFunctionType.Copy,
                                     scale=float(Cr / K)
                                     ).then_inc(S["s_vs"], 1)

        @block.vector
        def _(ve):
            import concourse.mybir as mb
            ve.wait_ge(io, 80)
            # phase A: even-chunk psum->sbuf copies (downcast to bf16)
            q = 0
            for g, (j0, n) in enumerate(groups):
                first = True
                for a in range(n):
                    if q % 2 == 0:
                        ve.wait_ge(S["s_u"], q + 1)
                        if first and g >= 2:
                            # usb[g%2] reuse: store of group g-2 done
                            ve.wait_ge(S["s_ust"][g % 2], 16 * (g // 2))
                            first = False
                        nc.vector.tensor_copy(usb[:, g % 2, a, :],
                                              ps_u[q % 2][:, 0:128]
                                              ).then_inc(S["s_uc"], 1)
                    q += 1
            # phase B: max(T, v) in bf16; then previous block's fixup
            for r in range(NB):
                Cr = int(LS[r] + HS[r])
                ve.wait_ge(S["s_ui"][r % UIB], 16 * (r // UIB + 1))
                if has_bias:
                    nc.vector.tensor_tensor(ui2[:, r % UIB, :],
                                            ui[:, r % UIB, :],
                                            br_sb[:],
                                            mb.AluOpType.subtract
                                            ).then_inc(S["s_uia"], 1)
                    nc.vector.drain()
                    usrc = ui2
                else:
                    usrc = ui   # b == 0: v = u_i directly
                ve.wait_ge(S["s_g"][r % TTB], 32 * (r // TTB + 1))
                nc.vector.tensor_tensor(
                    D_t[:, r % 2, 0:Cr, :], T_t[:, r % TTB, 0:Cr, :],
                    usrc[:, r % UIB, None, :].broadcast_to([128, Cr, 128]),
                    mb.AluOpType.max).then_inc(S["s_sub"], 1)
                # fixup for block r-1: o = ps_o*(1/K) - vs   (vs = v*Cr/K)
                if r >= 1:
                    _fixup(nc, S, ve, vs, o_t, ps_o, r - 1, OTB, VSB)
            _fixup(nc, S, ve, vs, o_t, ps_o, NB - 1, OTB, VSB)

        @block.gpsimd
        def _(gp):
            gp.load_library(mlp)
            gp.wait_ge(io, 80)
            s_ust, s_g = S["s_ust"], S["s_g"]
            gp.wait_ge(s_ust[0], 16 * UST_FINAL[0])
            gp.wait_ge(s_ust[1], 16 * UST_FINAL[1])
            for r in range(NB):
                LSr, HSr = int(LS[r]), int(HS[r])
                if r >= TTB:
                    gp.wait_ge(S["s_sub"], r - TTB + 1)    # T_t[r%TTB] reuse
                gp.dma_gather(
                    T_t[:, r % TTB, 0:LSr, :], tab_d[0:LO, :],
                    idx_sb[:, off_lo[r]:off_lo[r] + 8 * LSr],
                    128 * LSr, 128 * LSr, H,
                    single_packet=False, queue_num=r % NQ
                ).then_inc(s_g[r % TTB], 16)
                gp.dma_gather(
                    T_t[:, r % TTB, LSr:LSr + HSr, :],
                    tab_d[LO:R_TABLE, :],
                    idx_sb[:, off_hi[r]:off_hi[r] + 8 * HSr],
                    128 * HSr, 128 * HSr, H,
                    single_packet=False, queue_num=r % NQ
                ).then_inc(s_g[r % TTB], 16)

    nc.compile()
    return nc


def _fixup(nc, S, ve, vs, o_t, ps_o, r, OTB, VSB):
    import concourse.mybir as mb
    ve.wait_ge(S["s_acc"], r + 1)
    ve.wait_ge(S["s_vs"], r + 1)
    if r >= OTB:
        # o_t[r%OTB] reuse: store of block r-OTB done
        ve.wait_ge(S["s_out"][r % OTB], 16 * (r // OTB))
    nc.vector.scalar_tensor_tensor(
        o_t[:, r % OTB, :], ps_o[r % 2][:, 0:128], float(1.0 / K),
        vs[:, r % VSB, :], mb.AluOpType.mult, mb.AluOpType.subtract
    ).then_inc(S["s_red"], 1)


# ----------------------------------------------------------------- kernel()

def kernel(x, W, b, edge_index, _simulate=False):
    import ml_dtypes
    bf = ml_dtypes.bfloat16
    x = np.asarray(x, np.float32)
    W = np.asarray(W, np.float32)
    b = np.asarray(b, np.float32)
    edge_index = np.asarray(edge_index)

    cores, LS, HS, off_lo, off_hi, icols = _prep(edge_index)

    if _simulate:
        full = np.empty((N, H), np.float32)
        for c, co in enumerate(cores):
            out = _simulate_core(_xperm(x, co), W, b, co, LS, HS)
            full[co["sorted_ids"]] = out[:OWN]
        return full

    from concourse.bass_utils import run_bass_kernel_spmd

    has_bias = bool(np.any(b != 0))
    nc = _build_program(LS, HS, off_lo, off_hi, icols, has_bias)

    brep = np.tile(b[None, :], (128, 1)).astype(bf)
    ident = np.eye(128, dtype=np.float32).astype(bf)
    neg = np.full((1, H), BIG_NEG, np.float32).astype(bf)
    in_maps = []
    for co in cores:
        in_maps.append({
            "xpt": _xperm_t(x, co).astype(bf),
            "Wm": W.astype(bf),
            "brep": brep,
            "identm": ident,
            "negrow": neg,
            "idx": co["idx"],
        })

    trace = bool(int(os.environ.get("KERNEL_TRACE", "0")))
    res = run_bass_kernel_spmd(nc, in_maps, core_ids=list(range(NCORES)),
                               trace=trace)
    global LAST_RESULTS
    LAST_RESULTS = res

    full = np.empty((N, H), np.float32)
    for c, co in enumerate(cores):
        out = np.asarray(res.results[c]["out"], np.float32)
        full[co["sorted_ids"]] = out[:OWN]
    return full
